# revision 24
# baseline (speedup 1.0000x reference)
"""Trainium2 Bass kernel for nn_LocalDownsample (segment mean-pool via one-hot matmul).

Contract: kernel(**inputs) takes FULL inputs (x [8,4096,512] f32,
regions [8,4096] i64, max_n=512), returns FULL output [8,512,512] f32.

Sharding: pure data parallel — batch b -> core b. Per core:
  out[n-1, :] = mean over tokens t with regions[t] == n of x[t, :]   (0 if empty)

Device algorithm per core (T=4096 tokens, C=512 channels, N=512 regions),
mode "fp8dr" (default):
  tokens laid out as t = p*32 + j (p = SBUF partition, j = k-tile).
  Matmuls run in fp8e4m3 DoubleRow perf mode (0.5 cycles/row, 2x fp16):
  each DR matmul contracts a TOKEN PAIR (j, j+1) = 2x128 rows.  Precision
  comes from a two-term split x ~= hi + lo with hi = fp8(x),
  lo = fp8(x - hi): two DR streams (hi, lo) accumulate into the same PSUM
  banks -> ~1e-3 rel err (harness gate 2e-2).  Per pair: one-hot pair
  [P,2,NR] f8 (2 DVE is_equal), hi [P,2,C] (Act copy; merged into one op
  when the pair shares a DMA chunk), lo (DVE subtract, per-token ops --
  3-dim DVE APs and any gpsimd fp8 op are HW slow paths, avoid).  Counts
  ride DR matmuls vs a ones column padded to [P,2,16] (DR weights need
  even 16B-aligned k-subtile strides).  PE ~17us, DVE ~21us, Act ~19us,
  all under the ~23.5us x-DMA stream (the HW floor: 8 MiB fp32 at
  ~360 GB/s/core).  Tail: counts->recip->PE-transpose overlaps the final
  m-major matmuls; osb_m = acc[m] * rt[:, m] (Act scale), 4x256KiB DMA out.

mode "fp16": previous fp16 path (optionally with a residual split matmul).
"""

import numpy as np

import concourse.bacc as bacc
import concourse.bass as bass  # noqa: F401
import concourse.mybir as mybir
import concourse.tile as tile
from concourse.bass_utils import run_bass_kernel_spmd

P = 128          # SBUF partitions
T = 4096         # tokens per batch
C = 512          # channels
NR = 512         # number of regions (max_n)
JT = T // P      # 32 k-tiles
MC = NR // P     # 4 output row chunks
NCORES = 8

F8 = mybir.dt.float8e4
F16 = mybir.dt.float16
F32 = mybir.dt.float32
I32 = mybir.dt.int32

DEFAULT_CFG = dict(mode="fp8dr", chunks=(1, 1, 2, 4, 4, 4, 4, 4, 4, 2, 1, 1),
                   lo_pool_mod=0, dual_ring=False, merge_hi=False,
                   merge_lo=False, repeats=1)

_CACHE = {}


def _build_fp8dr(chunks, lo_pool_mod, dual_ring, merge_hi, merge_lo, repeats):
    assert sum(chunks) == JT
    DR = mybir.MatmulPerfMode.DoubleRow
    JP = JT // 2    # 16 token pairs; DoubleRow contracts (2 k-subtiles) x 128
    nc = bacc.Bacc(None, target_bir_lowering=False)
    x_d = nc.dram_tensor("x", [T, C], F32, kind="ExternalInput")
    r_d = nc.dram_tensor("regions", [T], I32, kind="ExternalInput")
    o_d = nc.dram_tensor("out", [NR, C], F32, kind="ExternalOutput")

    with tile.TileContext(nc) as tc:
        with (
            tc.tile_pool(name="const", bufs=1) as cpool,
            tc.tile_pool(name="xf", bufs=len(chunks)) as xf_pool,
            tc.tile_pool(name="xhi", bufs=JP) as xhi_pool,
            tc.tile_pool(name="xlo", bufs=JP) as xlo_pool,
            tc.tile_pool(name="oh", bufs=JP) as oh_pool,
            tc.tile_pool(name="eplg", bufs=1) as out_pool,
            tc.tile_pool(name="psum", bufs=1, space="PSUM") as psum_pool,
        ):
            # --- constants; regions ride the Activation HWDGE ring so the
            # SP ring belongs to the x stream from t=0 ---
            r_i = cpool.tile([P, JT], I32, tag="r_i")
            nc.scalar.dma_start(r_i[:], r_d.rearrange("(p j) -> p j", p=P))
            r_f = cpool.tile([P, JT], F32, tag="r_f")
            nc.vector.tensor_copy(r_f[:], r_i[:])

            iota16 = cpool.tile([P, NR], F16, tag="iota16")
            nc.gpsimd.iota(
                iota16[:], pattern=[[1, NR]], base=1, channel_multiplier=0,
                allow_small_or_imprecise_dtypes=True,  # 1..512 exact in fp16
            )

            # DR weights need even, 16B-aligned k-subtile strides: pad the
            # ones column out to [P, 2, 16] and slice [:, :, 0:1]
            ones8 = cpool.tile([P, 2, 16], F8, tag="ones8")
            nc.vector.memset(ones8[:], 1.0)
            ident1 = cpool.tile([1, 1], F32, tag="ident1")
            nc.vector.memset(ident1[:], 1.0)

            def body():
                # x stream: queue all chunk DMAs up front (FIFO on the SP ring,
                # ramped sizes so the first matmuls can start early)
                xv = x_d.rearrange("(p j) c -> p j c", p=P)
                xf = []          # per j: (chunk_tile, index within chunk)
                j0 = 0
                for ci, csz in enumerate(chunks):
                    t = xf_pool.tile([P, csz, C], F32, name=f"xfc{ci}", tag="xf")
                    ring = nc.scalar if (dual_ring and ci % 2) else nc.sync
                    ring.dma_start(t[:], xv[:, j0 : j0 + csz, :])
                    for jj in range(csz):
                        xf.append((t, jj))
                    j0 += csz

                # one PSUM bank per accumulation group: start=True clears
                # has_written for the whole bank
                acc = [
                    psum_pool.tile([P, C], F32, name=f"acc{m}", tag=f"acc{m}")
                    for m in range(MC)
                ]
                cnt = psum_pool.tile([1, NR], F32, tag="cnt")

                ohs = []     # per pair: [P, 2, NR] f8 one-hot pair (DR lhsT)
                xhis = []    # per pair: [P, 2, C] f8 hi = fp8(x)
                xlos = []    # per pair: [P, 2, C] f8 lo = fp8(x - hi)

                def conv(jp):
                    ohp = oh_pool.tile([P, 2, NR], F8, name=f"oh{jp}", tag="oh")
                    hip = xhi_pool.tile([P, 2, C], F8, name=f"xhi{jp}", tag="xhi")
                    lop = xlo_pool.tile([P, 2, C], F8, name=f"xlo{jp}", tag="xlo")
                    for k in range(2):
                        j = 2 * jp + k
                        nc.vector.tensor_scalar(
                            out=ohp[:, k, :],
                            in0=iota16[:],
                            scalar1=r_f[:, j : j + 1],
                            scalar2=None,
                            op0=mybir.AluOpType.is_equal,
                        )
                    # merged variants need both pair tokens in one chunk tile
                    xt, jj = xf[2 * jp]
                    same_chunk = xf[2 * jp + 1][0] is xt and xf[2 * jp + 1][1] == jj + 1
                    lo_eng = (
                        nc.gpsimd
                        if (lo_pool_mod and jp % lo_pool_mod == lo_pool_mod - 1)
                        else nc.vector
                    )
                    if merge_hi and same_chunk:
                        nc.scalar.copy(hip[:], xt[:, jj : jj + 2, :])
                    else:
                        for k in range(2):
                            xt_k, jj_k = xf[2 * jp + k]
                            nc.scalar.copy(hip[:, k, :], xt_k[:, jj_k, :])
                    if merge_lo and same_chunk:
                        lo_eng.tensor_tensor(
                            out=lop[:], in0=xt[:, jj : jj + 2, :], in1=hip[:],
                            op=mybir.AluOpType.subtract,
                        )
                    else:
                        for k in range(2):
                            xt_k, jj_k = xf[2 * jp + k]
                            lo_eng.tensor_tensor(
                                out=lop[:, k, :], in0=xt_k[:, jj_k, :],
                                in1=hip[:, k, :],
                                op=mybir.AluOpType.subtract,
                            )
                    ohs.append(ohp)
                    xhis.append(hip)
                    xlos.append(lop)
                    # counts accumulate on the PE
                    nc.tensor.matmul(
                        cnt[:], lhsT=ones8[:, :, 0:1], rhs=ohp[:],
                        start=(jp == 0), stop=(jp == JP - 1),
                        perf_mode=DR, skip_group_check=True,
                    )

                def mm(m, jp):
                    for rhs, first, last in (
                        (xhis[jp], jp == 0, False),
                        (xlos[jp], False, jp == JP - 1),
                    ):
                        nc.tensor.matmul(
                            acc[m][:],
                            lhsT=ohs[jp][:, :, m * P : (m + 1) * P],
                            rhs=rhs[:],
                            start=first,
                            stop=last,
                            perf_mode=DR,
                            skip_group_check=True,
                        )

                LAST = JT - chunks[-1]     # trailing pairs run m-major
                tail = [jp for jp in range(JP) if 2 * jp + 1 >= LAST]
                for jp in range(JP):
                    conv(jp)
                    if jp not in tail:
                        for m in range(MC):
                            mm(m, jp)

                # counts -> reciprocal -> PE transpose to [128, 4]
                csb = out_pool.tile([1, NR], F32, tag="csb")
                nc.vector.tensor_scalar_max(csb[:], cnt[:], 1.0)
                recip = out_pool.tile([1, NR], F32, tag="recip")
                nc.vector.reciprocal(recip[:], csb[:])
                rt = out_pool.tile([P, MC], F32, tag="rt")
                for m in range(MC):
                    rp = psum_pool.tile([P, 1], F32, name=f"rp{m}", tag=f"rp{m % 2}")
                    nc.tensor.transpose(
                        rp[:], recip[:, m * P : (m + 1) * P], ident1[:]
                    )
                    nc.vector.tensor_copy(rt[:, m : m + 1], rp[:])

                osb = out_pool.tile([P, MC, C], F32, tag="osb")
                for m in range(MC):
                    for jp in tail:
                        mm(m, jp)
                    # Phase C (per m, overlaps later m's matmuls); Act engine
                    # applies the per-partition 1/count scale
                    nc.scalar.mul(osb[:, m, :], acc[m][:], rt[:, m : m + 1])
                    nc.sync.dma_start(o_d[m * P : (m + 1) * P, :], osb[:, m, :])

            if repeats == 1:
                body()
            else:
                with tc.For_i(0, repeats, 1, hint_engines=(mybir.EngineType.PE,)):
                    body()

    nc.compile()
    return nc


def _build_fp16(chunks, split, repeats):
    assert sum(chunks) == JT
    nc = bacc.Bacc(None, target_bir_lowering=False)
    x_d = nc.dram_tensor("x", [T, C], F32, kind="ExternalInput")
    r_d = nc.dram_tensor("regions", [T], I32, kind="ExternalInput")
    o_d = nc.dram_tensor("out", [NR, C], F32, kind="ExternalOutput")

    with tile.TileContext(nc) as tc:
        with (
            tc.tile_pool(name="const", bufs=1) as cpool,
            tc.tile_pool(name="xf", bufs=len(chunks)) as xf_pool,
            tc.tile_pool(name="x16", bufs=10) as x16_pool,
            tc.tile_pool(name="oh", bufs=JT) as oh_pool,
            tc.tile_pool(name="eplg", bufs=1) as out_pool,
            tc.tile_pool(name="psum", bufs=1, space="PSUM") as psum_pool,
        ):
            r_i = cpool.tile([P, JT], I32, tag="r_i")
            nc.scalar.dma_start(r_i[:], r_d.rearrange("(p j) -> p j", p=P))
            r_f = cpool.tile([P, JT], F32, tag="r_f")
            nc.vector.tensor_copy(r_f[:], r_i[:])

            iota16 = cpool.tile([P, NR], F16, tag="iota16")
            nc.gpsimd.iota(
                iota16[:], pattern=[[1, NR]], base=1, channel_multiplier=0,
                allow_small_or_imprecise_dtypes=True,
            )

            ones_st = cpool.tile([P, 1], F32, tag="ones_st")
            nc.vector.memset(ones_st[:], 1.0)
            ident1 = cpool.tile([1, 1], F32, tag="ident1")
            nc.vector.memset(ident1[:], 1.0)

            def body():
                xv = x_d.rearrange("(p j) c -> p j c", p=P)
                xf = []
                j0 = 0
                for ci, csz in enumerate(chunks):
                    t = xf_pool.tile([P, csz, C], F32, name=f"xfc{ci}", tag="xf")
                    nc.sync.dma_start(t[:], xv[:, j0 : j0 + csz, :])
                    for jj in range(csz):
                        xf.append((t, jj))
                    j0 += csz

                acc = [
                    psum_pool.tile([P, C], F32, name=f"acc{m}", tag=f"acc{m}")
                    for m in range(MC)
                ]
                cnt = psum_pool.tile([1, NR], F32, tag="cnt")

                oh = []
                oh_sum = out_pool.tile([P, NR], F32, tag="oh_sum")
                x16s = {}

                def load_x16(j):
                    xt, jj = xf[j]
                    x16 = x16_pool.tile([P, C], F16, name=f"x16_{j}", tag="x16")
                    if j < 2:
                        nc.vector.tensor_copy(x16[:], xt[:, jj, :])
                    else:
                        nc.scalar.copy(x16[:], xt[:, jj, :])
                    xlo = None
                    if split:
                        xlo = x16_pool.tile([P, C], F16, name=f"xlo_{j}", tag="xlo")
                        nc.vector.tensor_tensor(
                            out=xlo[:], in0=xt[:, jj, :], in1=x16[:],
                            op=mybir.AluOpType.subtract,
                        )
                    x16s[j] = (x16, xlo)

                def mm(m, j):
                    x16, xlo = x16s[j]
                    nc.tensor.matmul(
                        acc[m][:],
                        lhsT=oh[j][:, m * P : (m + 1) * P],
                        rhs=x16[:],
                        start=(j == 0),
                        stop=(j == JT - 1) and not split,
                        skip_group_check=True,
                    )
                    if split:
                        nc.tensor.matmul(
                            acc[m][:],
                            lhsT=oh[j][:, m * P : (m + 1) * P],
                            rhs=xlo[:],
                            start=False,
                            stop=(j == JT - 1),
                            skip_group_check=True,
                        )

                LAST = JT - chunks[-1]
                for j in range(JT):
                    t = oh_pool.tile([P, NR], F16, name=f"oh{j}", tag="oh")
                    nc.vector.tensor_scalar(
                        out=t[:],
                        in0=iota16[:],
                        scalar1=r_f[:, j : j + 1],
                        scalar2=None,
                        op0=mybir.AluOpType.is_equal,
                    )
                    oh.append(t)
                    load_x16(j)
                    if j == 0:
                        nc.vector.tensor_copy(oh_sum[:], t[:])
                    else:
                        nc.vector.tensor_tensor(
                            out=oh_sum[:], in0=oh_sum[:], in1=t[:],
                            op=mybir.AluOpType.add,
                        )
                    if j < LAST:
                        for m in range(MC):
                            mm(m, j)

                nc.tensor.matmul(
                    cnt[:], lhsT=ones_st[:], rhs=oh_sum[:],
                    start=True, stop=True, skip_group_check=True,
                )
                csb = out_pool.tile([1, NR], F32, tag="csb")
                nc.vector.tensor_scalar_max(csb[:], cnt[:], 1.0)
                recip = out_pool.tile([1, NR], F32, tag="recip")
                nc.vector.reciprocal(recip[:], csb[:])
                rt = out_pool.tile([P, MC], F32, tag="rt")
                for m in range(MC):
                    rp = psum_pool.tile([P, 1], F32, name=f"rp{m}", tag=f"rp{m % 2}")
                    nc.tensor.transpose(
                        rp[:], recip[:, m * P : (m + 1) * P], ident1[:]
                    )
                    nc.vector.tensor_copy(rt[:, m : m + 1], rp[:])

                osb = out_pool.tile([P, MC, C], F32, tag="osb")
                for m in range(MC):
                    for j in range(LAST, JT):
                        mm(m, j)
                    nc.vector.tensor_scalar(
                        out=osb[:, m, :],
                        in0=acc[m][:],
                        scalar1=rt[:, m : m + 1],
                        scalar2=None,
                        op0=mybir.AluOpType.mult,
                    )
                    nc.sync.dma_start(o_d[m * P : (m + 1) * P, :], osb[:, m, :])

            if repeats == 1:
                body()
            else:
                with tc.For_i(0, repeats, 1, hint_engines=(mybir.EngineType.PE,)):
                    body()

    nc.compile()
    return nc


def _build(mode="fp8dr", chunks=(1, 1, 2, 4, 4, 4, 4, 4, 4, 2, 1, 1),
           lo_pool_mod=0, dual_ring=False, merge_hi=False, merge_lo=False,
           split=False, repeats=1):
    if mode == "fp8dr":
        return _build_fp8dr(chunks, lo_pool_mod, dual_ring, merge_hi, merge_lo,
                            repeats)
    return _build_fp16(chunks, split, repeats)


def _get_nc(**cfg):
    cfg = {**DEFAULT_CFG, **cfg}
    key = tuple(sorted(cfg.items()))
    if key not in _CACHE:
        kw = dict(cfg)
        mode = kw.pop("mode")
        if mode == "fp8dr":
            kw.pop("split", None)
        else:
            kw.pop("lo_pool_mod", None)
            kw.pop("dual_ring", None)
            kw.pop("merge_hi", None)
            kw.pop("merge_lo", None)
        _CACHE[key] = _build(mode=mode, **kw)
    return _CACHE[key]


def kernel(x, regions, max_n, _trace=False, _tmpdir=None, _cfg=None):
    x = np.asarray(x, dtype=np.float32)
    regions = np.asarray(regions)
    assert x.shape == (NCORES, T, C), x.shape
    assert regions.shape == (NCORES, T), regions.shape
    assert int(np.asarray(max_n)) == NR

    r32 = np.ascontiguousarray(regions.astype(np.int32))

    nc = _get_nc(**(_cfg or {}))
    in_maps = [
        {"x": np.ascontiguousarray(x[b]), "regions": r32[b]} for b in range(NCORES)
    ]
    try:
        res = run_bass_kernel_spmd(
            nc,
            in_maps,
            core_ids=list(range(NCORES)),
            trace=_trace,
            tmpdir=_tmpdir,
        )
    except Exception:
        # one retry for transient runtime/tunnel failures
        res = run_bass_kernel_spmd(
            nc,
            in_maps,
            core_ids=list(range(NCORES)),
            trace=_trace,
            tmpdir=_tmpdir,
        )
    out = np.stack([res.results[b]["out"] for b in range(NCORES)], axis=0)
    if _trace:
        kernel._last_results = res
    return out
